# revision 15
# baseline (speedup 1.0000x reference)
# Triplet-margin loss kernel for Trainium2 (Bass/Tile), batch-sharded
# across 8 NeuronCores.
#
# reference math (torch F.pairwise_distance semantics):
#   d_ap[b,p] = || anc[b] - pos[b,p] + eps ||_2
#   d_an[b,n] = || anc[b] - neg[b,n] + eps ||_2
#   loss = mean_{b,p,n} max(d_ap[b,p] - d_an[b,n] + margin, 0)
#
# Per 128-row batch tile there are 24 distance columns ("slices"), each a
# [128, 1024] fp32 read. Every slice uses the same two-engine path:
#   dot a'.x on DVE (scalar_tensor_tensor fp32, accum_out) and ||x||^2 on
#   ACT (activation Square, accum_out); d = sqrt(nrm - 2 dot + ||a'||^2).
# With that split DVE (~33 us/tile), ACT (~34 us/tile) and DMA
# (~35 us/tile) are all at the roofline ridge.
# GpSimd is deliberately UNUSED: measurements show DVE ops stretch from
# ~1.2 us to ~3.2 us while GpSimd runs (shared SBUF ports), so any work
# routed through GpSimd lowers total elementwise throughput.
# Ramp/tail trims: the first two chunks are single slices so compute
# starts ~2.5 us earlier; a dummy Sqrt primes the ACT function table
# (Square and Sqrt share one table - avoids a 1.3 us mid-kernel load);
# the pos-column combine (d2c/sqrt/s_m) is split out so it runs as soon
# as the pos chunks finish; the (p,n) pairing is split 4 columns on DVE
# (stt subtract/min-0, negated sums) and 4 on ACT (Relu(s_p - d_an) with
# per-partition bias, positive sums) - the host fixes the signs.
# Each core returns per-partition partial sums [128, 2*NT]; the host sums
# and scales.

import numpy as np

import concourse.bacc as bacc
import concourse.mybir as mybir
import concourse.tile as tile
from concourse import bass_utils

B, Z = 2048, 1024
NUM_POS, NUM_NEG = 8, 16
NJ = NUM_POS + NUM_NEG
MARGIN, EPS = 1.0, 1e-6
N_CORES = 8
BL = B // N_CORES  # 256 rows of anc per core
P = 128
NT = BL // P  # 2 batch-tiles per core
CH = 1  # z-slices per full DMA chunk
XP_BUFS = 28
NPAIR_DVE = 5  # pos columns paired on DVE; the rest on ACT

F32 = mybir.dt.float32
BF16 = mybir.dt.bfloat16
AF = mybir.ActivationFunctionType
OP = mybir.AluOpType

# chunk list: (first_jj, n_slices), DMA-issued in this order; small lead
# chunks cut the ramp, split trailing chunks cut the tail.
CHUNKS = [(j, 1) for j in range(NJ)]


def _emit(tc, nc, anc, pos, neg, out):
    v = nc.vector
    act = nc.scalar
    pos2 = pos.rearrange("(b j) z -> b (j z)", j=NUM_POS)  # [BL, 8*Z]
    neg2 = neg.rearrange("(b j) z -> b (j z)", j=NUM_NEG)  # [BL, 16*Z]
    with (
        tc.tile_pool(name="xp", bufs=XP_BUFS) as xp,
        tc.tile_pool(name="apool", bufs=2) as apool,
        tc.tile_pool(name="scp", bufs=1) as scp,
        tc.tile_pool(name="smp", bufs=2) as smp,
        tc.tile_pool(name="pp", bufs=2, space="PSUM") as pp,
        tc.tile_pool(name="opool", bufs=1) as opool,
    ):
        osb = opool.tile([P, 2 * NT], F32, name="osb")
        dve_scr = scp.tile([P, Z], BF16, name="dve_scr")
        act_scr = scp.tile([P, Z], BF16, name="act_scr")
        ts_out = scp.tile([P, NUM_NEG], F32, name="ts_out")
        zero_n = opool.tile([P, NUM_NEG], F32, name="zero_n")
        v.memset(zero_n[:, :], 0.0)
        # prime the ACT function table with Sqrt so Square+Sqrt load once
        act.activation(ts_out[:, 0:1], zero_n[:, 0:1], AF.Sqrt)

        # prologue: both tiles' anc loads, a' = anc + eps, ||a'||^2
        aprimes, anrms = [], []
        for t in range(NT):
            b0 = t * P
            anc_in = apool.tile([P, Z], F32, name="anc_in")
            aprime = apool.tile([P, Z], F32, name="aprime")
            a_nrm = smp.tile([P, 1], F32, name="a_nrm")
            nc.scalar.dma_start(anc_in[:, :], anc[b0 : b0 + P, :])
            v.tensor_scalar_add(aprime[:, :], anc_in[:, :], EPS)
            v.scalar_tensor_tensor(
                out=dve_scr[:, :],
                in0=anc_in[:, :],
                scalar=1.0,
                in1=aprime[:, :],
                op0=OP.bypass,
                op1=OP.mult,
                accum_out=a_nrm[:, 0:1],
            )
            aprimes.append(aprime)
            anrms.append(a_nrm)

        for t in range(NT):
            b0 = t * P
            aprime = aprimes[t]
            a_nrm = anrms[t]
            dot = smp.tile([P, NJ], F32, name="dot")
            nrm = pp.tile([P, NJ], F32, name="nrm")
            d2c = smp.tile([P, NJ], F32, name="d2c")
            dt_ = smp.tile([P, NJ], F32, name="dt_")
            s_m = smp.tile([P, NUM_POS], F32, name="s_m")
            lp = smp.tile([P, NUM_POS], F32, name="lp")

            tiles = []
            for jj0, nsl in CHUNKS:
                w = nsl * Z
                xt = xp.tile([P, CH * Z], F32, name="xt")
                if jj0 < NUM_POS:
                    src = pos2[b0 : b0 + P, jj0 * Z : jj0 * Z + w]
                else:
                    src = neg2[
                        b0 : b0 + P, (jj0 - NUM_POS) * Z : (jj0 - NUM_POS) * Z + w
                    ]
                nc.sync.dma_start(xt[:, 0:w], src)
                tiles.append(xt)

            emitted_pos_combine = False
            for (jj0, nsl), xt in zip(CHUNKS, tiles):
                for q in range(nsl):
                    jj = jj0 + q
                    xs = xt[:, q * Z : (q + 1) * Z]
                    v.scalar_tensor_tensor(
                        out=dve_scr[:, :],
                        in0=xs,
                        scalar=1.0,
                        in1=aprime[:, :],
                        op0=OP.bypass,
                        op1=OP.mult,
                        accum_out=dot[:, jj : jj + 1],
                    )
                    act.activation(
                        act_scr[:, :], xs, AF.Square, accum_out=nrm[:, jj : jj + 1]
                    )
                if jj0 + nsl == NUM_POS and not emitted_pos_combine:
                    emitted_pos_combine = True
                    # pos cols complete: d_ap = sqrt(nrm - 2 dot + ||a'||^2),
                    # s = d_ap + margin
                    v.scalar_tensor_tensor(
                        out=d2c[:, 0:NUM_POS],
                        in0=dot[:, 0:NUM_POS],
                        scalar=-2.0,
                        in1=nrm[:, 0:NUM_POS],
                        op0=OP.mult,
                        op1=OP.add,
                    )
                    act.activation(
                        dt_[:, 0:NUM_POS],
                        d2c[:, 0:NUM_POS],
                        AF.Sqrt,
                        bias=a_nrm[:, 0:1],
                        scale=1.0,
                    )
                    v.tensor_scalar_add(s_m[:, :], dt_[:, 0:NUM_POS], MARGIN)

            # neg cols: d_an
            v.scalar_tensor_tensor(
                out=d2c[:, NUM_POS:NJ],
                in0=dot[:, NUM_POS:NJ],
                scalar=-2.0,
                in1=nrm[:, NUM_POS:NJ],
                op0=OP.mult,
                op1=OP.add,
            )
            act.activation(
                dt_[:, NUM_POS:NJ],
                d2c[:, NUM_POS:NJ],
                AF.Sqrt,
                bias=a_nrm[:, 0:1],
                scale=1.0,
            )
            # pairing: lp[:,p] for p < NPAIR_DVE holds -sum_n relu(s_p - d_an)
            # (DVE min-trick); for p >= NPAIR_DVE holds +sum_n relu(s_p - d_an)
            # (ACT Relu with bias). Host combines with per-column signs.
            for p_i in range(NPAIR_DVE):
                v.scalar_tensor_tensor(
                    out=ts_out[:, :],
                    in0=dt_[:, NUM_POS:NJ],
                    scalar=s_m[:, p_i : p_i + 1],
                    in1=zero_n[:, :],
                    op0=OP.subtract,
                    op1=OP.min,
                    accum_out=lp[:, p_i : p_i + 1],
                )
            for p_i in range(NPAIR_DVE, NUM_POS):
                act.activation(
                    ts_out[:, :],
                    dt_[:, NUM_POS:NJ],
                    AF.Relu,
                    bias=s_m[:, p_i : p_i + 1],
                    scale=-1.0,
                    accum_out=lp[:, p_i : p_i + 1],
                )
            v.reduce_sum(
                osb[:, 2 * t : 2 * t + 1], lp[:, 0:NPAIR_DVE], axis=mybir.AxisListType.X
            )
            v.reduce_sum(
                osb[:, 2 * t + 1 : 2 * t + 2],
                lp[:, NPAIR_DVE:NUM_POS],
                axis=mybir.AxisListType.X,
            )
            # earlier tiles' out-DMAs ride the idle GpSimd SWDGE ring: a
            # sync-ring trigger would wait on this tile's reduce and block the
            # next tile's chunk DMA triggers (FIFO per issuing engine). The
            # last tile has nothing after it, so use the faster HWDGE ring.
            if t < NT - 1:
                nc.gpsimd.dma_start(
                    out[:, 2 * t : 2 * t + 2], osb[:, 2 * t : 2 * t + 2]
                )
            else:
                nc.sync.dma_start(
                    out[:, 2 * t : 2 * t + 2], osb[:, 2 * t : 2 * t + 2]
                )


_NC_CACHE = None


def build():
    global _NC_CACHE
    if _NC_CACHE is None:
        nc = bacc.Bacc(
            "TRN2", target_bir_lowering=False, debug=False, num_devices=N_CORES
        )
        anc = nc.dram_tensor("anc", (BL, Z), F32, kind="ExternalInput").ap()
        pos = nc.dram_tensor("pos", (BL * NUM_POS, Z), F32, kind="ExternalInput").ap()
        neg = nc.dram_tensor("neg", (BL * NUM_NEG, Z), F32, kind="ExternalInput").ap()
        out = nc.dram_tensor("out", (P, 2 * NT), F32, kind="ExternalOutput").ap()
        with tile.TileContext(nc) as tc:
            _emit(tc, nc, anc, pos, neg, out)
        nc.compile()
        _NC_CACHE = nc
    return _NC_CACHE


def make_in_maps(anc_embedding, pos_embedding, neg_embedding):
    anc_embedding = np.asarray(anc_embedding, dtype=np.float32)
    pos_embedding = np.asarray(pos_embedding, dtype=np.float32)
    neg_embedding = np.asarray(neg_embedding, dtype=np.float32)
    in_maps = []
    for c in range(N_CORES):
        in_maps.append(
            {
                "anc": np.ascontiguousarray(anc_embedding[c * BL : (c + 1) * BL]),
                "pos": np.ascontiguousarray(
                    pos_embedding[c * BL * NUM_POS : (c + 1) * BL * NUM_POS]
                ),
                "neg": np.ascontiguousarray(
                    neg_embedding[c * BL * NUM_NEG : (c + 1) * BL * NUM_NEG]
                ),
            }
        )
    return in_maps


def combine(outs):
    # outs: list of [P, 2*NT]: even cols hold -sum relu (DVE pairing),
    # odd cols hold +sum relu (ACT pairing)
    total = 0.0
    for o in outs:
        o = o.astype(np.float64)
        total += o[:, 1::2].sum() - o[:, 0::2].sum()
    return np.float32(total / (B * NUM_POS * NUM_NEG))


def kernel(anc_embedding, pos_embedding, neg_embedding):
    nc = build()
    in_maps = make_in_maps(anc_embedding, pos_embedding, neg_embedding)
    res = bass_utils.run_bass_kernel_spmd(nc, in_maps, core_ids=list(range(N_CORES)))
    return combine([r["out"] for r in res.results])


# revision 16
# speedup vs baseline: 1.0031x; 1.0031x over previous
# Triplet-margin loss kernel for Trainium2 (Bass/Tile), batch-sharded
# across 8 NeuronCores.
#
# reference math (torch F.pairwise_distance semantics):
#   d_ap[b,p] = || anc[b] - pos[b,p] + eps ||_2
#   d_an[b,n] = || anc[b] - neg[b,n] + eps ||_2
#   loss = mean_{b,p,n} max(d_ap[b,p] - d_an[b,n] + margin, 0)
#
# Per 128-row batch tile there are 24 distance columns ("slices"), each a
# [128, 1024] fp32 read. Every slice uses the same two-engine path:
#   dot a'.x on DVE (scalar_tensor_tensor fp32, accum_out) and ||x||^2 on
#   ACT (activation Square, accum_out); d = sqrt(nrm - 2 dot + ||a'||^2).
# With that split DVE (~33 us/tile), ACT (~34 us/tile) and DMA
# (~35 us/tile) are all at the roofline ridge.
# GpSimd is deliberately UNUSED: measurements show DVE ops stretch from
# ~1.2 us to ~3.2 us while GpSimd runs (shared SBUF ports), so any work
# routed through GpSimd lowers total elementwise throughput.
# Ramp/tail trims: the first two chunks are single slices so compute
# starts ~2.5 us earlier; a dummy Sqrt primes the ACT function table
# (Square and Sqrt share one table - avoids a 1.3 us mid-kernel load);
# the pos-column combine (d2c/sqrt/s_m) is split out so it runs as soon
# as the pos chunks finish; the (p,n) pairing is split 4 columns on DVE
# (stt subtract/min-0, negated sums) and 4 on ACT (Relu(s_p - d_an) with
# per-partition bias, positive sums) - the host fixes the signs.
# Each core returns per-partition partial sums [128, 2*NT]; the host sums
# and scales.

import numpy as np

import concourse.bacc as bacc
import concourse.mybir as mybir
import concourse.tile as tile
from concourse import bass_utils

B, Z = 2048, 1024
NUM_POS, NUM_NEG = 8, 16
NJ = NUM_POS + NUM_NEG
MARGIN, EPS = 1.0, 1e-6
N_CORES = 8
BL = B // N_CORES  # 256 rows of anc per core
P = 128
NT = BL // P  # 2 batch-tiles per core
CH = 2  # z-slices per full DMA chunk
XP_BUFS = 16
NPAIR_DVE = 5  # pos columns paired on DVE; the rest on ACT

F32 = mybir.dt.float32
BF16 = mybir.dt.bfloat16
AF = mybir.ActivationFunctionType
OP = mybir.AluOpType

# chunk list: (first_jj, n_slices), DMA-issued in this order; small lead
# chunks cut the ramp, split trailing chunks cut the tail.
CHUNKS = (
    [(j, 1) for j in range(6)]
    + [(j, 2) for j in range(6, 22, 2)]
    + [(22, 1), (23, 1)]
)


def _emit(tc, nc, anc, pos, neg, out):
    v = nc.vector
    act = nc.scalar
    pos2 = pos.rearrange("(b j) z -> b (j z)", j=NUM_POS)  # [BL, 8*Z]
    neg2 = neg.rearrange("(b j) z -> b (j z)", j=NUM_NEG)  # [BL, 16*Z]
    with (
        tc.tile_pool(name="xp", bufs=XP_BUFS) as xp,
        tc.tile_pool(name="apool", bufs=2) as apool,
        tc.tile_pool(name="scp", bufs=1) as scp,
        tc.tile_pool(name="smp", bufs=2) as smp,
        tc.tile_pool(name="pp", bufs=2, space="PSUM") as pp,
        tc.tile_pool(name="opool", bufs=1) as opool,
    ):
        osb = opool.tile([P, 2 * NT], F32, name="osb")
        dve_scr = scp.tile([P, Z], BF16, name="dve_scr")
        act_scr = scp.tile([P, Z], BF16, name="act_scr")
        ts_out = scp.tile([P, NUM_NEG], F32, name="ts_out")
        zero_n = opool.tile([P, NUM_NEG], F32, name="zero_n")
        v.memset(zero_n[:, :], 0.0)
        # prime the ACT function table with Sqrt so Square+Sqrt load once
        act.activation(ts_out[:, 0:1], zero_n[:, 0:1], AF.Sqrt)

        # prologue: both tiles' anc loads, a' = anc + eps, ||a'||^2
        aprimes, anrms = [], []
        for t in range(NT):
            b0 = t * P
            anc_in = apool.tile([P, Z], F32, name="anc_in")
            aprime = apool.tile([P, Z], F32, name="aprime")
            a_nrm = smp.tile([P, 1], F32, name="a_nrm")
            nc.scalar.dma_start(anc_in[:, :], anc[b0 : b0 + P, :])
            v.tensor_scalar_add(aprime[:, :], anc_in[:, :], EPS)
            v.scalar_tensor_tensor(
                out=dve_scr[:, :],
                in0=anc_in[:, :],
                scalar=1.0,
                in1=aprime[:, :],
                op0=OP.bypass,
                op1=OP.mult,
                accum_out=a_nrm[:, 0:1],
            )
            aprimes.append(aprime)
            anrms.append(a_nrm)

        for t in range(NT):
            b0 = t * P
            aprime = aprimes[t]
            a_nrm = anrms[t]
            dot = smp.tile([P, NJ], F32, name="dot")
            nrm = pp.tile([P, NJ], F32, name="nrm")
            d2c = smp.tile([P, NJ], F32, name="d2c")
            dt_ = smp.tile([P, NJ], F32, name="dt_")
            s_m = smp.tile([P, NUM_POS], F32, name="s_m")
            lp = smp.tile([P, NUM_POS], F32, name="lp")

            tiles = []
            for jj0, nsl in CHUNKS:
                w = nsl * Z
                xt = xp.tile([P, CH * Z], F32, name="xt")
                if jj0 < NUM_POS:
                    src = pos2[b0 : b0 + P, jj0 * Z : jj0 * Z + w]
                else:
                    src = neg2[
                        b0 : b0 + P, (jj0 - NUM_POS) * Z : (jj0 - NUM_POS) * Z + w
                    ]
                nc.sync.dma_start(xt[:, 0:w], src)
                tiles.append(xt)

            emitted_pos_combine = False
            for (jj0, nsl), xt in zip(CHUNKS, tiles):
                for q in range(nsl):
                    jj = jj0 + q
                    xs = xt[:, q * Z : (q + 1) * Z]
                    v.scalar_tensor_tensor(
                        out=dve_scr[:, :],
                        in0=xs,
                        scalar=1.0,
                        in1=aprime[:, :],
                        op0=OP.bypass,
                        op1=OP.mult,
                        accum_out=dot[:, jj : jj + 1],
                    )
                    act.activation(
                        act_scr[:, :], xs, AF.Square, accum_out=nrm[:, jj : jj + 1]
                    )
                if jj0 + nsl == NUM_POS and not emitted_pos_combine:
                    emitted_pos_combine = True
                    # pos cols complete: d_ap = sqrt(nrm - 2 dot + ||a'||^2),
                    # s = d_ap + margin
                    v.scalar_tensor_tensor(
                        out=d2c[:, 0:NUM_POS],
                        in0=dot[:, 0:NUM_POS],
                        scalar=-2.0,
                        in1=nrm[:, 0:NUM_POS],
                        op0=OP.mult,
                        op1=OP.add,
                    )
                    act.activation(
                        dt_[:, 0:NUM_POS],
                        d2c[:, 0:NUM_POS],
                        AF.Sqrt,
                        bias=a_nrm[:, 0:1],
                        scale=1.0,
                    )
                    v.tensor_scalar_add(s_m[:, :], dt_[:, 0:NUM_POS], MARGIN)

            # neg cols: d_an
            v.scalar_tensor_tensor(
                out=d2c[:, NUM_POS:NJ],
                in0=dot[:, NUM_POS:NJ],
                scalar=-2.0,
                in1=nrm[:, NUM_POS:NJ],
                op0=OP.mult,
                op1=OP.add,
            )
            act.activation(
                dt_[:, NUM_POS:NJ],
                d2c[:, NUM_POS:NJ],
                AF.Sqrt,
                bias=a_nrm[:, 0:1],
                scale=1.0,
            )
            # pairing: lp[:,p] for p < NPAIR_DVE holds -sum_n relu(s_p - d_an)
            # (DVE min-trick); for p >= NPAIR_DVE holds +sum_n relu(s_p - d_an)
            # (ACT Relu with bias). Host combines with per-column signs.
            for p_i in range(NPAIR_DVE):
                v.scalar_tensor_tensor(
                    out=ts_out[:, :],
                    in0=dt_[:, NUM_POS:NJ],
                    scalar=s_m[:, p_i : p_i + 1],
                    in1=zero_n[:, :],
                    op0=OP.subtract,
                    op1=OP.min,
                    accum_out=lp[:, p_i : p_i + 1],
                )
            for p_i in range(NPAIR_DVE, NUM_POS):
                act.activation(
                    ts_out[:, :],
                    dt_[:, NUM_POS:NJ],
                    AF.Relu,
                    bias=s_m[:, p_i : p_i + 1],
                    scale=-1.0,
                    accum_out=lp[:, p_i : p_i + 1],
                )
            v.reduce_sum(
                osb[:, 2 * t : 2 * t + 1], lp[:, 0:NPAIR_DVE], axis=mybir.AxisListType.X
            )
            v.reduce_sum(
                osb[:, 2 * t + 1 : 2 * t + 2],
                lp[:, NPAIR_DVE:NUM_POS],
                axis=mybir.AxisListType.X,
            )
            # earlier tiles' out-DMAs ride the idle GpSimd SWDGE ring: a
            # sync-ring trigger would wait on this tile's reduce and block the
            # next tile's chunk DMA triggers (FIFO per issuing engine). The
            # last tile has nothing after it, so use the faster HWDGE ring.
            if t < NT - 1:
                nc.gpsimd.dma_start(
                    out[:, 2 * t : 2 * t + 2], osb[:, 2 * t : 2 * t + 2]
                )
            else:
                nc.sync.dma_start(
                    out[:, 2 * t : 2 * t + 2], osb[:, 2 * t : 2 * t + 2]
                )


_NC_CACHE = None


def build():
    global _NC_CACHE
    if _NC_CACHE is None:
        nc = bacc.Bacc(
            "TRN2", target_bir_lowering=False, debug=False, num_devices=N_CORES
        )
        anc = nc.dram_tensor("anc", (BL, Z), F32, kind="ExternalInput").ap()
        pos = nc.dram_tensor("pos", (BL * NUM_POS, Z), F32, kind="ExternalInput").ap()
        neg = nc.dram_tensor("neg", (BL * NUM_NEG, Z), F32, kind="ExternalInput").ap()
        out = nc.dram_tensor("out", (P, 2 * NT), F32, kind="ExternalOutput").ap()
        with tile.TileContext(nc) as tc:
            _emit(tc, nc, anc, pos, neg, out)
        nc.compile()
        _NC_CACHE = nc
    return _NC_CACHE


def make_in_maps(anc_embedding, pos_embedding, neg_embedding):
    anc_embedding = np.asarray(anc_embedding, dtype=np.float32)
    pos_embedding = np.asarray(pos_embedding, dtype=np.float32)
    neg_embedding = np.asarray(neg_embedding, dtype=np.float32)
    in_maps = []
    for c in range(N_CORES):
        in_maps.append(
            {
                "anc": np.ascontiguousarray(anc_embedding[c * BL : (c + 1) * BL]),
                "pos": np.ascontiguousarray(
                    pos_embedding[c * BL * NUM_POS : (c + 1) * BL * NUM_POS]
                ),
                "neg": np.ascontiguousarray(
                    neg_embedding[c * BL * NUM_NEG : (c + 1) * BL * NUM_NEG]
                ),
            }
        )
    return in_maps


def combine(outs):
    # outs: list of [P, 2*NT]: even cols hold -sum relu (DVE pairing),
    # odd cols hold +sum relu (ACT pairing)
    total = 0.0
    for o in outs:
        o = o.astype(np.float64)
        total += o[:, 1::2].sum() - o[:, 0::2].sum()
    return np.float32(total / (B * NUM_POS * NUM_NEG))


def kernel(anc_embedding, pos_embedding, neg_embedding):
    nc = build()
    in_maps = make_in_maps(anc_embedding, pos_embedding, neg_embedding)
    res = bass_utils.run_bass_kernel_spmd(nc, in_maps, core_ids=list(range(N_CORES)))
    return combine([r["out"] for r in res.results])
